# revision 23
# baseline (speedup 1.0000x reference)
"""DiffusionDet head on 8 Trainium2 cores (Bass/Tile, SPMD).

Strategy (data-parallel over boxes):
- Host: per-box FPN level + pooling rect; separable bilinear weights folded
  into a dense (pixels x 49) matrix A per box; pixels packed into 128-pixel
  chunks with per-slot chunk budgets balanced across cores so one SPMD
  program serves all 8 cores. Features are transposed to NHWC and
  concatenated per image so each pooled pixel is one contiguous row.
- Device: indirect-DMA gather of pixel chunks + matmul pooling -> roi;
  per-image-group AllGather of pooled features for self-attention; dynamic
  conv (per-box p1/p2 from a streamed weight), FFN, cls/reg heads, box
  delta decode. Large streamed weights and gathered features in fp16
  (fp32 accumulate); the fp32 path uses single-pass fp32r matmuls.
"""
import math
from contextlib import ExitStack

import numpy as np

import concourse.bass as bass
import concourse.bacc as bacc
import concourse.mybir as mybir
import concourse.tile as tile
from concourse.bass_utils import run_bass_kernel_spmd
from concourse.masks import make_identity

D = 256
HEADS = 8
HD = D // HEADS
DFF = 2048
R = 7
SR = 2
NB = R * R  # 49 bins
NUM_CLASSES = 80
DIM_DYN = 64
N_IMG = 2
NR = 300
IMG = 800
STRIDES = (4, 8, 16, 32)
SCALE_CLAMP = math.log(1000.0 / 16)
HWS = [(IMG // s, IMG // s) for s in STRIDES]
LEVEL_BASE = [0]
for _h, _w in HWS[:-1]:
    LEVEL_BASE.append(LEVEL_BASE[-1] + _h * _w)
NPIX_TOT = sum(h * w for h, w in HWS)
NCORES = 8
B = N_IMG * NR // NCORES  # 75 boxes per core
CHUNK = 128
AGRP = 32  # A-matrix chunks loaded per DMA group
NPAR = 2 * D * DIM_DYN  # 32768
EPS = 1e-5

f32 = mybir.dt.float32
f16 = mybir.dt.float16
i32 = mybir.dt.int32
AF = mybir.ActivationFunctionType
ALU = mybir.AluOpType
AX = mybir.AxisListType


# --------------------------------------------------------------------------
# host-side prep
# --------------------------------------------------------------------------

def _assign_levels(boxes):
    area = (boxes[:, 2] - boxes[:, 0]) * (boxes[:, 3] - boxes[:, 1])
    lv = np.floor(4.0 + np.log2(np.sqrt(area) / 224.0 + 1e-8))
    return np.clip(lv, 2.0, 5.0).astype(np.int32) - 2


def _box_plan(box, lv):
    H, W = HWS[lv]
    scale = 1.0 / STRIDES[lv]
    x1 = box[0] * scale - 0.5
    y1 = box[1] * scale - 0.5
    bw = (box[2] - box[0]) * scale / R
    bh = (box[3] - box[1]) * scale / R
    off = (np.arange(R * SR, dtype=np.float64) + 0.5) / SR
    gx = x1 + off * bw
    gy = y1 + off * bh
    validx = (gx >= -1.0) & (gx <= W)
    validy = (gy >= -1.0) & (gy <= H)
    x = np.clip(gx, 0.0, W - 1.0)
    y = np.clip(gy, 0.0, H - 1.0)
    x0 = np.floor(x)
    y0 = np.floor(y)
    lx = x - x0
    ly = y - y0
    x0i = x0.astype(np.int64)
    y0i = y0.astype(np.int64)
    x1i = np.minimum(x0i + 1, W - 1)
    y1i = np.minimum(y0i + 1, H - 1)
    x_lo = int(x0i.min()); x_hi = int(x1i.max())
    y_lo = int(y0i.min()); y_hi = int(y1i.max())
    wx = x_hi - x_lo + 1
    wy = y_hi - y_lo + 1
    Ax = np.zeros((wx, R), np.float64)
    Ay = np.zeros((wy, R), np.float64)
    for s in range(R * SR):
        b = s // SR
        if validx[s]:
            Ax[x0i[s] - x_lo, b] += (1.0 - lx[s]) / SR
            Ax[x1i[s] - x_lo, b] += lx[s] / SR
        if validy[s]:
            Ay[y0i[s] - y_lo, b] += (1.0 - ly[s]) / SR
            Ay[y1i[s] - y_lo, b] += ly[s] / SR
    A = np.einsum('yk,xl->yxkl', Ay, Ax).reshape(wy * wx, NB)
    ys, xs = np.meshgrid(np.arange(y_lo, y_hi + 1), np.arange(x_lo, x_hi + 1),
                         indexing='ij')
    pix = (LEVEL_BASE[lv] + ys * W + xs).reshape(-1)
    return pix.astype(np.int32), A.astype(np.float32)


def _host_prep(bboxes):
    M = N_IMG * NR
    boxes = np.asarray(bboxes, np.float32).reshape(M, 4)
    lv = _assign_levels(boxes.astype(np.float64))
    plans = [_box_plan(boxes[m].astype(np.float64), int(lv[m]))
             for m in range(M)]
    ncb = np.array([(len(p[0]) + CHUNK - 1) // CHUNK for p in plans])
    assign = [[None] * B for _ in range(NCORES)]
    for img in range(N_IMG):
        idxs = np.arange(img * NR, (img + 1) * NR)
        order = idxs[np.argsort(-ncb[idxs], kind='stable')]
        for rank, m in enumerate(order):
            assign[4 * img + rank % 4][rank // 4] = int(m)
    ncs = np.zeros(B, np.int64)
    for c in range(NCORES):
        for s in range(B):
            ncs[s] = max(ncs[s], ncb[assign[c][s]])
    return plans, assign, ncs, boxes


def _pack_core(plans, assign_core, ncs):
    TOTCH = int(ncs.sum())
    idx = np.zeros((TOTCH, CHUNK), np.int32)
    A_pack = np.zeros((TOTCH, CHUNK, NB), np.float32)
    off = 0
    for s in range(B):
        pix, A = plans[assign_core[s]]
        n = len(pix)
        nch = int(ncs[s])
        fi = np.zeros(nch * CHUNK, np.int32)
        fA = np.zeros((nch * CHUNK, NB), np.float32)
        fi[:n] = pix
        fA[:n] = A
        idx[off:off + nch] = fi.reshape(nch, CHUNK)
        A_pack[off:off + nch] = fA.reshape(nch, CHUNK, NB)
        off += nch
    # device layouts: idx (128, TOTCH); A (128, TOTCH, 49) fp16
    return (np.ascontiguousarray(idx.T),
            np.ascontiguousarray(
                A_pack.transpose(1, 0, 2).astype(np.float16)))


# --------------------------------------------------------------------------
# device program
# --------------------------------------------------------------------------

def _build_program(ncs):
    TOTCH = int(ncs.sum())
    nc = bacc.Bacc(num_devices=NCORES)

    def din(name, shape, dt=f32):
        return nc.dram_tensor(name, list(shape), dt, kind="ExternalInput")

    def dout(name, shape, dt=f32):
        return nc.dram_tensor(name, list(shape), dt, kind="ExternalOutput")

    feat = din("feat", (NPIX_TOT, D), f16)
    idx_in = din("idx", (CHUNK, TOTCH), i32)
    apack = din("apack", (CHUNK, TOTCH, NB), f16)
    bbox_in = din("bbox", (B, 4))
    temb_in = din("temb", (1024,))
    wiT_in = din("wiT", (D, 3 * D))       # attn_Wi.T
    biq_in = din("biq", (32, HEADS))      # q bias, per-head columns
    bik_in = din("bik", (32, HEADS))      # k bias, per-head columns
    biv_in = din("biv", (1, D))           # v bias row
    woT_in = din("woT", (32, HEADS, D))   # attn_Wo.T reshaped (dd, h, c)
    bo_in = din("bo", (1, D))
    dynWt_in = din("dynWt", (D, NPAR), f16)
    dynb_in = din("dynb", (1, NPAR))
    iioutT_in = din("iioutT", (NB * D, D), f16)
    iioutb_in = din("iioutb", (1, D))
    lin1T_in = din("lin1T", (D, DFF), f16)
    lin1b_in = din("lin1b", (1, DFF))
    lin2T_in = din("lin2T", (DFF, D), f16)
    lin2b_in = din("lin2b", (1, D))
    timeWT_in = din("timeWT", (4 * D, 2 * D))
    timeb_in = din("timeb", (1, 2 * D))
    clsWT_in = din("clsWT", (D, D))
    regWT_in = [din("regWT%d" % r, (D, D)) for r in range(3)]
    clslogWT_in = din("clslogWT", (D, NUM_CLASSES))
    clslogb_in = din("clslogb", (1, NUM_CLASSES))
    deltaWT_in = din("deltaWT", (D, 4))
    deltab_in = din("deltab", (1, 4))

    logits_out = dout("logits", (B, NUM_CLASSES))
    pred_out = dout("pred", (B, 4))
    obj_out = dout("obj", (B, D))

    with tile.TileContext(nc) as tc, ExitStack() as ctx:
        consts = ctx.enter_context(tc.tile_pool(name="consts", bufs=1))
        big = ctx.enter_context(tc.tile_pool(name="big", bufs=1))
        work = ctx.enter_context(tc.tile_pool(name="work", bufs=3))
        patchp = ctx.enter_context(tc.tile_pool(name="patchp", bufs=6))
        apool = ctx.enter_context(tc.tile_pool(name="apool", bufs=2))
        wstream = ctx.enter_context(tc.tile_pool(name="wstream", bufs=3))
        pp = ctx.enter_context(tc.tile_pool(name="pp", bufs=6, space="PSUM"))
        pacc = ctx.enter_context(tc.tile_pool(name="pacc", bufs=2,
                                              space="PSUM"))
        dram = ctx.enter_context(tc.tile_pool(name="dram", bufs=1,
                                              space="DRAM"))

        ident = consts.tile([128, 128], f32)
        make_identity(nc, ident)
        ident16 = consts.tile([128, 128], f16)
        nc.vector.tensor_copy(ident16, ident)
        eps_t = consts.tile([128, 1], f32)
        nc.vector.memset(eps_t, EPS)
        ones_r = consts.tile([1, 128], f32)
        nc.vector.memset(ones_r, 1.0)

        def bias_mm(ps, row_ap, P):
            # += ones(1,P).T @ row(1,N) : broadcast row-add via K=1 matmul
            nc.tensor.matmul(ps, lhsT=ones_r[:, :P],
                             rhs=row_ap.bitcast(f32),
                             start=False, stop=True)

        def transpose(in_ap):
            """PE transpose (P, F) -> psum (F, P), storage dtype f16/f32."""
            P, F = in_ap.shape
            dt_store = f16 if in_ap.dtype == f16 else f32
            t_full = pp.tile([128, 128], dt_store, tag="ps", name="tp_ps")
            t = t_full[:F, :P]
            if in_ap.dtype == f16:
                nc.tensor.transpose(t, in_ap, ident16[:P, :P])
            else:
                nc.tensor.transpose(t, in_ap.bitcast(f32), ident[:P, :P])
            return t

        def ln_rows(dst, src, relu=False):
            """LayerNorm over free dim on (P, n); src may be PSUM."""
            P = src.shape[0]
            st_full = work.tile([128, 6], f32, tag="bnst", name="bnst")
            st = st_full[:P]
            mv_full = work.tile([128, 2], f32, tag="bnmv", name="bnmv")
            mv = mv_full[:P]
            nc.vector.bn_stats(out=st, in_=src)
            nc.vector.bn_aggr(out=mv, in_=st)
            sd_full = work.tile([128, 1], f32, tag="sd", name="sd")
            sd = sd_full[:P]
            nc.scalar.activation(out=sd, in_=mv[:, 1:2], func=AF.Sqrt,
                                 bias=eps_t[:P], scale=1.0)
            rs_full = work.tile([128, 1], f32, tag="lnrs", name="lnrs")
            rs = rs_full[:P]
            nc.vector.reciprocal(out=rs, in_=sd)
            nc.vector.tensor_scalar(out=dst, in0=src, scalar1=mv[:, 0:1],
                                    scalar2=rs, op0=ALU.subtract, op1=ALU.mult)
            if relu:
                nc.scalar.activation(out=dst, in_=dst, func=AF.Relu)

        def rowload(src, tag, dt=f32):
            t = consts.tile([1, src.shape[-1]], dt, name=tag)
            nc.sync.dma_start(out=t, in_=src[:].bitcast(dt))
            return t

        # ---------------- constants to SBUF ----------------
        idx_sb = consts.tile([CHUNK, TOTCH], i32)
        nc.sync.dma_start(out=idx_sb, in_=idx_in[:])
        wiT_sb = consts.tile([128, 2, 3 * D], f32)
        nc.sync.dma_start(out=wiT_sb,
                          in_=wiT_in.rearrange("(h p) n -> p h n", p=128).bitcast(f32))
        woT_sb = consts.tile([32, HEADS, D], f32)
        nc.sync.dma_start(out=woT_sb, in_=woT_in[:].bitcast(f32))
        biq_sb = consts.tile([32, HEADS], f32)
        nc.sync.dma_start(out=biq_sb, in_=biq_in[:])
        bik_sb = consts.tile([32, HEADS], f32)
        nc.sync.dma_start(out=bik_sb, in_=bik_in[:])
        biv_sb = rowload(biv_in, "biv")
        bo_sb = rowload(bo_in, "bo")
        iioutb_sb = rowload(iioutb_in, "iioutb")
        lin1b_sb = rowload(lin1b_in, "lin1b")
        lin2b_sb = rowload(lin2b_in, "lin2b")
        timeb_sb = rowload(timeb_in, "timeb")
        clsWT_sb = consts.tile([128, 2, D], f32)
        nc.sync.dma_start(out=clsWT_sb,
                          in_=clsWT_in.rearrange("(h p) n -> p h n", p=128).bitcast(f32))
        regWT_sb = []
        for r in range(3):
            t = consts.tile([128, 2, D], f32, name="regWT%d" % r)
            nc.sync.dma_start(out=t,
                              in_=regWT_in[r].rearrange("(h p) n -> p h n",
                                                        p=128).bitcast(f32))
            regWT_sb.append(t)
        clslogWT_sb = consts.tile([128, 2, NUM_CLASSES], f32)
        nc.sync.dma_start(out=clslogWT_sb,
                          in_=clslogWT_in.rearrange("(h p) n -> p h n", p=128).bitcast(f32))
        clslogb_sb = rowload(clslogb_in, "clslogb")
        deltaWT_sb = consts.tile([128, 2, 4], f32)
        nc.sync.dma_start(out=deltaWT_sb,
                          in_=deltaWT_in.rearrange("(h p) n -> p h n", p=128).bitcast(f32))
        deltab_sb = rowload(deltab_in, "deltab")
        bbox_sb = consts.tile([B, 4], f32)
        nc.sync.dma_start(out=bbox_sb, in_=bbox_in[:])

        # ---------------- phase 1: pooling ----------------
        roiT16 = big.tile([128, 2, B, NB], f16)
        proT = big.tile([128, 2, B], f32)
        a_sb = None
        a_base = -1
        off = 0
        for s in range(B):
            nch = int(ncs[s])
            roi_ps = pacc.tile([NB, D], f32, tag="acc", name="roi_ps")
            for k in range(nch):
                ch = off + k
                if ch // AGRP != a_base:
                    a_base = ch // AGRP
                    g = min(AGRP, TOTCH - a_base * AGRP)
                    a_sb = apool.tile([CHUNK, AGRP, NB], f16, tag="agrp",
                                      name="a_sb")
                    nc.sync.dma_start(
                        out=a_sb[:, :g, :],
                        in_=apack[:, a_base * AGRP:a_base * AGRP + g, :])
                pt = patchp.tile([CHUNK, D], f16, tag="patch", name="pt")
                nc.gpsimd.indirect_dma_start(
                    out=pt, out_offset=None, in_=feat[:],
                    in_offset=bass.IndirectOffsetOnAxis(
                        ap=idx_sb[:, ch:ch + 1], axis=0))
                nc.tensor.matmul(roi_ps,
                                 lhsT=a_sb[:, ch - a_base * AGRP, :],
                                 rhs=pt,
                                 start=(k == 0), stop=(k == nch - 1))
            roi_sb = work.tile([NB, D], f32, tag="roi_sb", name="roi_sb")
            nc.vector.tensor_copy(out=roi_sb, in_=roi_ps)
            for h in (0, 1):
                tp = transpose(roi_sb[:, h * 128:(h + 1) * 128])
                nc.vector.tensor_copy(out=roiT16[:, h, s, :], in_=tp)
                with nc.allow_low_precision(reason="f32 pro; consumed by "
                                            "f32 matmuls which round anyway"):
                    nc.vector.reduce_sum(out=proT[:, h, s:s + 1], in_=tp,
                                         axis=AX.X)
            off += nch
        nc.scalar.mul(out=proT[:], in_=proT[:], mul=1.0 / NB)

        # ---------------- phase 2: allgather pro ----------------
        cc_in = dram.tile([2, 128, B], f32)
        for h in (0, 1):
            nc.sync.dma_start(out=cc_in[h], in_=proT[:, h, :].bitcast(f32))
        cc_out = dram.tile([4, 2, 128, B], f32)
        nc.gpsimd.collective_compute(
            "AllGather", ALU.bypass,
            replica_groups=[[0, 1, 2, 3], [4, 5, 6, 7]],
            ins=[cc_in.opt()], outs=[cc_out.opt()])
        proT_img = big.tile([128, 2, 4, B], f32)
        for h in (0, 1):
            for g in range(4):
                nc.sync.dma_start(out=proT_img[:, h, g, :],
                                  in_=cc_out[g, h].bitcast(f32))

        pro_loc = big.tile([B, D], f32)
        for h in (0, 1):
            tp = transpose(proT[:, h, :])
            nc.vector.tensor_copy(out=pro_loc[:, h * 128:(h + 1) * 128],
                                  in_=tp)

        # ---------------- phase 3: attention ----------------
        qT = big.tile([32, HEADS, B], f16)
        kT = big.tile([32, HEADS, 4 * B], f16)
        for h in range(HEADS):
            ps32_full = pp.tile([32, 4 * B], f32, tag="ps", name="qk_ps")
            ps = ps32_full[:, :B]
            for kc in (0, 1):
                nc.tensor.matmul(ps,
                                 lhsT=wiT_sb[:, kc, h * HD:(h + 1) * HD].bitcast(f32),
                                 rhs=proT[:, kc, :].bitcast(f32),
                                 start=(kc == 0), stop=(kc == 1))
            nc.vector.tensor_scalar(out=qT[:, h, :], in0=ps,
                                    scalar1=biq_sb[:, h:h + 1], scalar2=None,
                                    op0=ALU.add)
            ps2_full = pp.tile([32, 4 * B], f32, tag="ps", name="k_ps")
            ps2 = ps2_full[:, :]
            for kc in (0, 1):
                nc.tensor.matmul(
                    ps2,
                    lhsT=wiT_sb[:, kc, D + h * HD:D + (h + 1) * HD].bitcast(f32),
                    rhs=proT_img[:, kc, :, :].rearrange("p g b -> p (g b)").bitcast(f32),
                    start=(kc == 0), stop=(kc == 1))
            nc.vector.tensor_scalar(out=kT[:, h, :], in0=ps2,
                                    scalar1=bik_sb[:, h:h + 1], scalar2=None,
                                    op0=ALU.add)
        vbm = big.tile([128, 3, D], f16)  # V box-major, 3 chunks of 128
        for mc in range(3):
            cnt = min(128, 4 * B - mc * 128)
            ps_full = pp.tile([128, D], f32, tag="ps", name="v_ps")
            ps = ps_full[:cnt]
            for kc in (0, 1):
                nc.tensor.matmul(
                    ps,
                    lhsT=proT_img[:, kc, :, :].rearrange("p g b -> p (g b)")[:, mc * 128:mc * 128 + cnt].bitcast(f32),
                    rhs=wiT_sb[:, kc, 2 * D:3 * D].bitcast(f32),
                    start=(kc == 0), stop=False)
            bias_mm(ps, biv_sb, cnt)
            nc.vector.tensor_copy(out=vbm[:cnt, mc, :], in_=ps)
        oT = big.tile([32, HEADS, B], f32)
        isq = 1.0 / math.sqrt(HD)
        for h in range(HEADS):
            s_ps = pp.tile([B, 4 * B], f32, tag="ps", name="s_ps")
            nc.tensor.matmul(s_ps, lhsT=qT[:, h, :], rhs=kT[:, h, :],
                             start=True, stop=True)
            mx = work.tile([B, 1], f32, tag="mx", name="mx")
            nc.vector.reduce_max(out=mx, in_=s_ps, axis=AX.X)
            nmx = work.tile([B, 1], f32, tag="nmx", name="nmx")
            nc.vector.tensor_scalar(out=nmx, in0=mx, scalar1=-isq,
                                    scalar2=None, op0=ALU.mult)
            a_t = work.tile([B, 4 * B], f32, tag="a_t", name="a_t")
            nc.scalar.activation(out=a_t, in_=s_ps, func=AF.Exp,
                                 bias=nmx, scale=isq)
            sm = work.tile([B, 1], f32, tag="sm", name="sm")
            nc.vector.reduce_sum(out=sm, in_=a_t, axis=AX.X)
            rcp = work.tile([B, 1], f32, tag="smr", name="rcp")
            nc.vector.reciprocal(out=rcp, in_=sm)
            nc.vector.tensor_scalar(out=a_t, in0=a_t, scalar1=rcp,
                                    scalar2=None, op0=ALU.mult)
            o_ps = pp.tile([32, B], f32, tag="ps", name="o_ps")
            for mc in range(3):
                cnt = min(128, 4 * B - mc * 128)
                aT = transpose(a_t[:, mc * 128:mc * 128 + cnt])
                a16_full = work.tile([128, B], f16, tag="a16", name="a16")
                a16 = a16_full[:cnt]
                nc.vector.tensor_copy(out=a16, in_=aT)
                nc.tensor.matmul(o_ps, lhsT=vbm[:cnt, mc, h * HD:(h + 1) * HD],
                                 rhs=a16,
                                 start=(mc == 0), stop=(mc == 2))
            nc.vector.tensor_copy(out=oT[:, h, :], in_=o_ps)
        at_ps = pp.tile([B, D], f32, tag="ps", name="at_ps")
        for h in range(HEADS):
            nc.tensor.matmul(at_ps, lhsT=oT[:, h, :].bitcast(f32),
                             rhs=woT_sb[:, h, :].bitcast(f32),
                             start=(h == 0), stop=False)
        bias_mm(at_ps, bo_sb, B)
        x_sb = big.tile([B, D], f32)  # pro1
        nc.vector.tensor_add(out=x_sb, in0=at_ps, in1=pro_loc)
        ln_rows(x_sb, x_sb)
        pro1T = big.tile([128, 2, B], f16)
        for h in (0, 1):
            tp = transpose(x_sb[:, h * 128:(h + 1) * 128])
            nc.vector.tensor_copy(out=pro1T[:, h, :], in_=tp)

        # ---------------- phase 4: dynamic params ----------------
        p1_all = big.tile([128, 2, B, DIM_DYN], f16)
        p2_dram = dram.tile([DIM_DYN, B, D], f16)
        for j in range(NPAR // 512):
            w_sb = wstream.tile([128, 2, 512], f16, tag="dynw", name="w_sb")
            nc.sync.dma_start(
                out=w_sb,
                in_=dynWt_in[:, j * 512:(j + 1) * 512].rearrange(
                    "(h p) n -> p h n", p=128))
            ps = pacc.tile([B, 512], f32, tag="acc", name="par_ps")
            for h in (0, 1):
                nc.tensor.matmul(ps, lhsT=pro1T[:, h, :],
                                 rhs=w_sb[:, h, :],
                                 start=(h == 0), stop=False)
            b_sb = wstream.tile([1, 512], f32, tag="dynb", name="b_sb")
            nc.sync.dma_start(out=b_sb,
                              in_=dynb_in[:, j * 512:(j + 1) * 512].bitcast(f32))
            bias_mm(ps, b_sb, B)
            stg = wstream.tile([B, 512], f16, tag="stg", name="stg")
            nc.vector.tensor_copy(out=stg, in_=ps)
            if j < 32:
                for c8 in range(8):
                    c = j * 8 + c8
                    nc.sync.dma_start(
                        out=p1_all[c % 128:c % 128 + 1, c // 128, :, :],
                        in_=stg[:, c8 * DIM_DYN:(c8 + 1) * DIM_DYN])
            else:
                for e2 in range(2):
                    e = (j - 32) * 2 + e2
                    nc.sync.dma_start(out=p2_dram[e, :, :],
                                      in_=stg[:, e2 * D:(e2 + 1) * D])

        # ---------------- phase 5: per-box dynamic conv ----------------
        ffT = big.tile([128, NB, 2, B], f16)
        for s in range(B):
            f1_ps = pp.tile([NB, DIM_DYN], f32, tag="ps", name="f1_ps")
            for h in (0, 1):
                nc.tensor.matmul(f1_ps, lhsT=roiT16[:, h, s, :],
                                 rhs=p1_all[:, h, s, :],
                                 start=(h == 0), stop=(h == 1))
            f1h = work.tile([NB, DIM_DYN], f16, tag="f1h", name="f1h")
            ln_rows(f1h, f1_ps, relu=True)
            f1T_ps = transpose(f1h)
            f1T = work.tile([DIM_DYN, NB], f16, tag="f1T", name="f1T")
            nc.vector.tensor_copy(out=f1T, in_=f1T_ps)
            p2_sb = work.tile([DIM_DYN, D], f16, tag="p2sb", name="p2_sb")
            nc.sync.dma_start(out=p2_sb, in_=p2_dram[:, s, :])
            f2_ps = pp.tile([NB, D], f32, tag="ps", name="f2_ps")
            nc.tensor.matmul(f2_ps, lhsT=f1T, rhs=p2_sb,
                             start=True, stop=True)
            f2h = work.tile([NB, D], f16, tag="f2h", name="f2h")
            ln_rows(f2h, f2_ps, relu=True)
            for h in (0, 1):
                tp = transpose(f2h[:, h * 128:(h + 1) * 128])
                nc.vector.tensor_copy(out=ffT[:, :, h, s], in_=tp)

        # ---------------- phase 6: ii_out + norm2 ----------------
        ii_ps = pacc.tile([B, D], f32, tag="acc", name="ii_ps")
        for k2 in range(2 * NB):
            w_sb = wstream.tile([128, D], f16, tag="iiw", name="iiw_sb")
            nc.sync.dma_start(out=w_sb,
                              in_=iioutT_in[k2 * 128:(k2 + 1) * 128, :])
            nc.tensor.matmul(ii_ps, lhsT=ffT[:, k2 // 2, k2 % 2, :],
                             rhs=w_sb,
                             start=(k2 == 0), stop=False)
        bias_mm(ii_ps, iioutb_sb, B)
        dcv = work.tile([B, D], f32, tag="dcv", name="dcv")
        ln_rows(dcv, ii_ps, relu=True)
        obj = big.tile([B, D], f32)
        nc.vector.tensor_add(out=obj, in0=x_sb, in1=dcv)
        ln_rows(obj, obj)

        # ---------------- phase 7: FFN + norm3 ----------------
        objT = work.tile([128, 2, B], f16, tag="objT", name="objT")
        for h in (0, 1):
            tp = transpose(obj[:, h * 128:(h + 1) * 128])
            nc.vector.tensor_copy(out=objT[:, h, :], in_=tp)
        hT = big.tile([128, DFF // 128, B], f16)
        for nck in range(DFF // 512):
            w_sb = wstream.tile([128, 2, 512], f16, tag="dynw", name="l1w")
            nc.sync.dma_start(
                out=w_sb,
                in_=lin1T_in[:, nck * 512:(nck + 1) * 512].rearrange(
                    "(h p) n -> p h n", p=128))
            ps = pacc.tile([B, 512], f32, tag="acc", name="h_ps")
            for h in (0, 1):
                nc.tensor.matmul(ps, lhsT=objT[:, h, :],
                                 rhs=w_sb[:, h, :],
                                 start=(h == 0), stop=False)
            bias_mm(ps, lin1b_sb[:, nck * 512:(nck + 1) * 512], B)
            h512 = work.tile([B, 512], f32, tag="h512", name="h512")
            nc.scalar.activation(out=h512, in_=ps, func=AF.Relu)
            for k in range(4):
                tp = transpose(h512[:, k * 128:(k + 1) * 128])
                nc.vector.tensor_copy(out=hT[:, nck * 4 + k, :], in_=tp)
        ffn_ps = pacc.tile([B, D], f32, tag="acc", name="ffn_ps")
        for k in range(DFF // 128):
            w_sb = wstream.tile([128, D], f16, tag="iiw", name="l2w")
            nc.sync.dma_start(out=w_sb, in_=lin2T_in[k * 128:(k + 1) * 128, :])
            nc.tensor.matmul(ffn_ps, lhsT=hT[:, k, :], rhs=w_sb,
                             start=(k == 0), stop=False)
        bias_mm(ffn_ps, lin2b_sb, B)
        nc.vector.tensor_add(out=obj, in0=obj, in1=ffn_ps)
        ln_rows(obj, obj)
        nc.sync.dma_start(out=obj_out[:], in_=obj.bitcast(f32))

        # ---------------- phase 8: scale/shift + heads ----------------
        sT = work.tile([128, 8], f32, tag="sT", name="sT")
        nc.sync.dma_start(out=sT,
                          in_=temb_in.rearrange("(k p) -> p k", p=128).bitcast(f32))
        nc.scalar.activation(out=sT, in_=sT, func=AF.Silu)
        ss_ps = pp.tile([1, 2 * D], f32, tag="ps", name="ss_ps")
        for k in range(8):
            w_sb = wstream.tile([128, 2 * D], f32, tag="dynw", name="tw")
            nc.sync.dma_start(out=w_sb,
                              in_=timeWT_in[k * 128:(k + 1) * 128, :].bitcast(f32))
            nc.tensor.matmul(ss_ps, lhsT=sT[:, k:k + 1].bitcast(f32),
                             rhs=w_sb.bitcast(f32),
                             start=(k == 0), stop=(k == 7))
        ss = work.tile([1, 2 * D], f32, tag="ss", name="ss")
        nc.vector.tensor_tensor(out=ss, in0=ss_ps, in1=timeb_sb, op=ALU.add)
        nc.vector.tensor_scalar(out=ss[:, :D], in0=ss[:, :D], scalar1=1.0,
                                scalar2=None, op0=ALU.add)
        ss_dram = dram.tile([1, 2 * D], f32)
        nc.sync.dma_start(out=ss_dram, in_=ss)
        scb = work.tile([B, D], f32, tag="scb", name="scb")
        nc.gpsimd.dma_start(out=scb, in_=ss_dram[:, :D].to_broadcast([B, D]))
        shb = work.tile([B, D], f32, tag="shb", name="shb")
        nc.gpsimd.dma_start(out=shb, in_=ss_dram[:, D:].to_broadcast([B, D]))
        fc = big.tile([B, D], f32)
        nc.vector.tensor_tensor(out=fc, in0=obj, in1=scb, op=ALU.mult)
        nc.vector.tensor_tensor(out=fc, in0=fc, in1=shb, op=ALU.add)

        def head_layer(src, wT_sb, tag):
            """relu(ln(src @ W.T)) box-major; returns new (B, D) tile."""
            sT_l = work.tile([128, 2, B], f32, tag="headT", name=tag + "T")
            for h in (0, 1):
                tp = transpose(src[:, h * 128:(h + 1) * 128])
                nc.vector.tensor_copy(out=sT_l[:, h, :], in_=tp)
            ps = pp.tile([B, D], f32, tag="ps", name=tag + "_ps")
            for h in (0, 1):
                nc.tensor.matmul(ps, lhsT=sT_l[:, h, :].bitcast(f32),
                                 rhs=wT_sb[:, h, :].bitcast(f32),
                                 start=(h == 0), stop=(h == 1))
            out_t = work.tile([B, D], f32, tag="heado", name=tag + "o")
            ln_rows(out_t, ps, relu=True)
            return out_t

        cf = head_layer(fc, clsWT_sb, "cls")
        rf = fc
        for r in range(3):
            rf = head_layer(rf, regWT_sb[r], "reg%d" % r)

        cfT = work.tile([128, 2, B], f32, tag="cfT", name="cfT")
        for h in (0, 1):
            tp = transpose(cf[:, h * 128:(h + 1) * 128])
            nc.vector.tensor_copy(out=cfT[:, h, :], in_=tp)
        lg_ps = pp.tile([B, NUM_CLASSES], f32, tag="ps", name="lg_ps")
        for h in (0, 1):
            nc.tensor.matmul(lg_ps, lhsT=cfT[:, h, :].bitcast(f32),
                             rhs=clslogWT_sb[:, h, :].bitcast(f32),
                             start=(h == 0), stop=False)
        bias_mm(lg_ps, clslogb_sb, B)
        lg = work.tile([B, NUM_CLASSES], f32, tag="lg", name="lg")
        nc.vector.tensor_copy(out=lg, in_=lg_ps)
        nc.sync.dma_start(out=logits_out[:], in_=lg)

        rfT = work.tile([128, 2, B], f32, tag="rfT", name="rfT")
        for h in (0, 1):
            tp = transpose(rf[:, h * 128:(h + 1) * 128])
            nc.vector.tensor_copy(out=rfT[:, h, :], in_=tp)
        dl_ps = pp.tile([B, 4], f32, tag="ps", name="dl_ps")
        for h in (0, 1):
            nc.tensor.matmul(dl_ps, lhsT=rfT[:, h, :].bitcast(f32),
                             rhs=deltaWT_sb[:, h, :].bitcast(f32),
                             start=(h == 0), stop=False)
        bias_mm(dl_ps, deltab_sb, B)
        dl = work.tile([B, 4], f32, tag="dl", name="dl")
        nc.vector.tensor_copy(out=dl, in_=dl_ps)

        # apply_deltas
        def col(t, j):
            return t[:, j:j + 1]

        tmp = work.tile([B, 12], f32, tag="adtmp", name="adtmp")
        wid, hei, cx, cy, pcx, pcy, pw, ph_, t0, t1 = (col(tmp, j)
                                                       for j in range(10))
        nc.vector.tensor_tensor(out=wid, in0=col(bbox_sb, 2),
                                in1=col(bbox_sb, 0), op=ALU.subtract)
        nc.vector.tensor_tensor(out=hei, in0=col(bbox_sb, 3),
                                in1=col(bbox_sb, 1), op=ALU.subtract)
        nc.vector.tensor_scalar(out=t0, in0=wid, scalar1=0.5, scalar2=None,
                                op0=ALU.mult)
        nc.vector.tensor_tensor(out=cx, in0=col(bbox_sb, 0), in1=t0,
                                op=ALU.add)
        nc.vector.tensor_scalar(out=t0, in0=hei, scalar1=0.5, scalar2=None,
                                op0=ALU.mult)
        nc.vector.tensor_tensor(out=cy, in0=col(bbox_sb, 1), in1=t0,
                                op=ALU.add)
        nc.vector.tensor_scalar(out=t0, in0=col(dl, 0), scalar1=0.5,
                                scalar2=None, op0=ALU.mult)
        nc.vector.tensor_tensor(out=t0, in0=t0, in1=wid, op=ALU.mult)
        nc.vector.tensor_tensor(out=pcx, in0=t0, in1=cx, op=ALU.add)
        nc.vector.tensor_scalar(out=t0, in0=col(dl, 1), scalar1=0.5,
                                scalar2=None, op0=ALU.mult)
        nc.vector.tensor_tensor(out=t0, in0=t0, in1=hei, op=ALU.mult)
        nc.vector.tensor_tensor(out=pcy, in0=t0, in1=cy, op=ALU.add)
        nc.vector.tensor_scalar(out=t0, in0=col(dl, 2), scalar1=SCALE_CLAMP,
                                scalar2=None, op0=ALU.min)
        nc.scalar.activation(out=t0, in_=t0, func=AF.Exp)
        nc.vector.tensor_tensor(out=pw, in0=t0, in1=wid, op=ALU.mult)
        nc.vector.tensor_scalar(out=t0, in0=col(dl, 3), scalar1=SCALE_CLAMP,
                                scalar2=None, op0=ALU.min)
        nc.scalar.activation(out=t0, in_=t0, func=AF.Exp)
        nc.vector.tensor_tensor(out=ph_, in0=t0, in1=hei, op=ALU.mult)
        pred_sb = work.tile([B, 4], f32, tag="pred", name="pred_sb")
        nc.vector.tensor_scalar(out=t0, in0=pw, scalar1=0.5, scalar2=None,
                                op0=ALU.mult)
        nc.vector.tensor_tensor(out=col(pred_sb, 0), in0=pcx, in1=t0,
                                op=ALU.subtract)
        nc.vector.tensor_tensor(out=col(pred_sb, 2), in0=pcx, in1=t0,
                                op=ALU.add)
        nc.vector.tensor_scalar(out=t1, in0=ph_, scalar1=0.5, scalar2=None,
                                op0=ALU.mult)
        nc.vector.tensor_tensor(out=col(pred_sb, 1), in0=pcy, in1=t1,
                                op=ALU.subtract)
        nc.vector.tensor_tensor(out=col(pred_sb, 3), in0=pcy, in1=t1,
                                op=ALU.add)
        nc.sync.dma_start(out=pred_out[:], in_=pred_sb)

    nc.compile()
    return nc


# --------------------------------------------------------------------------
# entry point
# --------------------------------------------------------------------------

def kernel(f2, f3, f4, f5, bboxes, time_emb, params):
    p = params
    f2, f3, f4, f5 = (np.asarray(t, np.float32) for t in (f2, f3, f4, f5))
    bboxes = np.asarray(bboxes, np.float32)
    time_emb = np.asarray(time_emb, np.float32)

    plans, assign, ncs, boxes_all = _host_prep(bboxes)
    nc = _build_program(ncs)

    feat_img = []
    for i in range(N_IMG):
        feat_img.append(np.ascontiguousarray(np.concatenate(
            [np.asarray(t[i]).transpose(1, 2, 0).reshape(-1, D)
             for t in (f2, f3, f4, f5)], axis=0).astype(np.float16)))

    def T(a):
        return np.ascontiguousarray(np.asarray(a, np.float32).T)

    def T16(a):
        return np.ascontiguousarray(
            np.asarray(a, np.float32).T.astype(np.float16))

    def row(a):
        return np.asarray(a, np.float32).reshape(1, -1)

    bi = np.asarray(p['attn_bi'], np.float32)
    shared = {
        "wiT": T(p['attn_Wi']),
        "biq": np.ascontiguousarray(bi[:D].reshape(HEADS, 32).T),
        "bik": np.ascontiguousarray(bi[D:2 * D].reshape(HEADS, 32).T),
        "biv": row(bi[2 * D:]),
        "woT": np.ascontiguousarray(
            T(p['attn_Wo']).reshape(HEADS, 32, D).transpose(1, 0, 2)),
        "bo": row(p['attn_bo']),
        "dynWt": T16(p['dyn_W']),
        "dynb": row(p['dyn_b']),
        "iioutT": T16(p['ii_out_W']),
        "iioutb": row(p['ii_out_b']),
        "lin1T": T16(p['lin1_W']),
        "lin1b": row(p['lin1_b']),
        "lin2T": T16(p['lin2_W']),
        "lin2b": row(p['lin2_b']),
        "timeWT": T(p['time_W']),
        "timeb": row(p['time_b']),
        "clsWT": T(p['cls_W'][0]),
        "clslogWT": T(p['clslog_W']),
        "clslogb": row(p['clslog_b']),
        "deltaWT": T(p['delta_W']),
        "deltab": row(p['delta_b']),
        "temb_dummy": None,
    }
    del shared["temb_dummy"]
    for r in range(3):
        shared["regWT%d" % r] = T(p['reg_W'][r])
    # LN affines are identity (ones/zeros) in this module by construction.
    for k in ('norm1', 'norm2', 'norm3', 'ii_n1', 'ii_n2', 'ii_n3'):
        assert np.allclose(np.asarray(p[k + '_g']), 1.0), k
        assert np.allclose(np.asarray(p[k + '_b']), 0.0), k
    for lst, val in (('cls_g', 1.0), ('reg_g', 1.0), ('cls_bn', 0.0),
                     ('reg_bn', 0.0)):
        for v in p[lst]:
            assert np.allclose(np.asarray(v), val), lst

    in_maps = []
    for c in range(NCORES):
        idx, apack = _pack_core(plans, assign[c], ncs)
        m = dict(shared)
        m["feat"] = feat_img[c // 4]
        m["idx"] = idx
        m["apack"] = apack
        m["bbox"] = np.ascontiguousarray(boxes_all[assign[c]])
        m["temb"] = time_emb[c // 4]
        in_maps.append(m)

    res = run_bass_kernel_spmd(nc, in_maps, list(range(NCORES)))

    logits = np.zeros((N_IMG * NR, NUM_CLASSES), np.float32)
    pred = np.zeros((N_IMG * NR, 4), np.float32)
    obj = np.zeros((N_IMG * NR, D), np.float32)
    for c in range(NCORES):
        r = res.results[c]
        for s in range(B):
            m = assign[c][s]
            logits[m] = r["logits"][s]
            pred[m] = r["pred"][s]
            obj[m] = r["obj"][s]
    return (logits.reshape(N_IMG, NR, NUM_CLASSES),
            pred.reshape(N_IMG, NR, 4),
            obj.reshape(1, N_IMG * NR, D))
